# revision 4
# baseline (speedup 1.0000x reference)
"""AdditiveAttention (Bahdanau) TRN2 Bass kernel.

softmax(mask ? tanh(vW + MU) @ v : -inf)  over rows, for
B=32, R=4096, D=1024, data-parallel over batch across 8 NeuronCores.

Per core (4 batches):
  - load W/U/v once, cast to fp16 (DVE); proj_v = vec @ W via PE (fp16)
    with vec transposed on PE.
  - per (batch, 1024-row block): load matrix rows fp32, PE-transpose
    128x128 tiles into PSUM (fp32), DVE-copy-cast to fp16 [d, r] layout,
    8 e-chunk matmul groups (8 fp16 matmuls each) -> PSUM fp32,
    tanh+bias on ScalarE -> fp16 inter, v-dot matmuls -> scores [1, r].
  - per batch: predicated-copy scores over a -100 background (mask),
    exp with fused accumulate -> softmax, DMA out fp32.
"""

from contextlib import ExitStack

import numpy as np

import bass_rust
import concourse.bass as bass
import concourse.tile as tile
from concourse import mybir
from concourse import bass_utils

F32 = mybir.dt.float32
F16 = mybir.dt.float16
I32 = mybir.dt.int32

B, R, D = 32, 4096, 1024
NCORES = 8
BPC = B // NCORES          # batches per core
RBLK = 1024                # rows per block
NBLK = R // RBLK           # blocks per batch
NT = RBLK // 128           # 128-row subtiles per block
NC_ = D // 128             # d (and e) chunks
NEG = -100.0               # masked logit; exp(-100) underflows to ~0 in fp32

_uid = [0]


def _legalize_waits(nc):
    """This walrus accepts at most 1 sync wait per instruction (2 for
    EventSemaphore); Tile's kernel-tail drain piles all terminal waits onto
    one Drain. Split the excess into wait-only EventSemaphores."""
    for f in nc.m.functions:
        for bb in f.blocks:
            insts = list(bb.instructions)
            new_insts = []
            changed = False
            for inst in insts:
                si = inst.sync_info
                waits = list(si.on_wait) if si is not None else []
                cap = 2 if isinstance(inst, mybir.InstEventSemaphore) else 1
                if len(waits) > cap:
                    changed = True
                    keep, rest = waits[:cap], waits[cap:]
                    for i in range(0, len(rest), 2):
                        _uid[0] += 1
                        ev = mybir.InstEventSemaphore(
                            name=f"lw_{inst.name}_{_uid[0]}", ins=[], outs=[]
                        )
                        ev.engine = inst.engine
                        ev.sync_info = bass_rust.SyncInfo(
                            on_wait=list(rest[i : i + 2]), on_update=[]
                        )
                        new_insts.append(ev)
                    inst.sync_info = bass_rust.SyncInfo(
                        on_wait=keep, on_update=list(si.on_update)
                    )
                new_insts.append(inst)
            if changed:
                bb.instructions = new_insts
    return nc


def _emit(nc, passes=1):
    vec_in = nc.dram_tensor("vec", [BPC, D], F32, kind="ExternalInput").ap()
    mat_in = nc.dram_tensor("mat", [BPC, R, D], F32, kind="ExternalInput").ap()
    mask_in = nc.dram_tensor("mask", [BPC, R], I32, kind="ExternalInput").ap()
    w_in = nc.dram_tensor("w", [D, D], F32, kind="ExternalInput").ap()
    u_in = nc.dram_tensor("u", [D, D], F32, kind="ExternalInput").ap()
    v_in = nc.dram_tensor("v", [D, 1], F32, kind="ExternalInput").ap()
    id_in = nc.dram_tensor("ident", [128, 128], F32, kind="ExternalInput").ap()
    out = nc.dram_tensor("out", [BPC, R], F32, kind="ExternalOutput").ap()

    with tile.TileContext(nc) as tc, ExitStack() as ctx:
        consts = ctx.enter_context(tc.tile_pool(name="consts", bufs=1))
        big = ctx.enter_context(tc.tile_pool(name="big", bufs=2))       # [128,8192]f32
        matT_p = ctx.enter_context(tc.tile_pool(name="matT", bufs=2))   # [128,8,1024]f16
        inter_p = ctx.enter_context(tc.tile_pool(name="inter", bufs=3))  # [128,1024]f16
        row_p = ctx.enter_context(tc.tile_pool(name="row", bufs=2))     # [1,4096]f32
        tp_ps = ctx.enter_context(tc.tile_pool(name="tp_ps", bufs=2, space="PSUM"))
        pm_ps = ctx.enter_context(tc.tile_pool(name="pm_ps", bufs=2, space="PSUM"))
        sc_ps = ctx.enter_context(tc.tile_pool(name="sc_ps", bufs=2, space="PSUM"))

        # ---------------- preamble: constants ----------------
        ident = consts.tile([128, 128], F32, tag="ident")
        nc.sync.dma_start(ident[:], id_in[:])

        # U -> fp16 [p, c, e]
        u32 = big.tile([128, NC_, D], F32, tag="big")
        nc.sync.dma_start(u32[:], u_in.rearrange("(c p) e -> p c e", p=128))
        u16 = consts.tile([128, NC_, D], F16, tag="u16")
        nc.vector.tensor_copy(u16[:], u32[:])

        # W -> fp16 [p, c, e]
        w32 = big.tile([128, NC_, D], F32, tag="big")
        nc.sync.dma_start(w32[:], w_in.rearrange("(c p) e -> p c e", p=128))
        w16 = consts.tile([128, NC_, D], F16, tag="w16")
        nc.vector.tensor_copy(w16[:], w32[:])

        # v -> fp16 [p, c]
        v32 = consts.tile([128, NC_], F32, tag="v32")
        nc.sync.dma_start(v32[:], v_in.rearrange("(c p) one -> p (c one)", p=128))
        v16 = consts.tile([128, NC_], F16, tag="v16")
        nc.vector.tensor_copy(v16[:], v32[:])

        # vec [BPC, D] -> vecT16 [p, c, b]
        vec_sb = consts.tile([BPC, D], F32, tag="vec")
        nc.sync.dma_start(vec_sb[:], vec_in[:])
        vecT16 = consts.tile([128, NC_, BPC], F16, tag="vecT")
        for c in range(NC_):
            tp = tp_ps.tile([128, 512], F32, tag="tp")
            nc.tensor.transpose(tp[:, 0:BPC], vec_sb[:, 128 * c : 128 * (c + 1)],
                                ident[0:BPC, 0:BPC])
            nc.vector.tensor_copy(vecT16[:, c, :], tp[:, 0:BPC])

        # proj_v[e, b] = sum_d W[d, e] vec[b, d]   (fp16 matmul, fp32 psum)
        pv_sb = consts.tile([128, NC_, BPC], F32, tag="pv")
        for k in range(NC_):
            pv = pm_ps.tile([128, RBLK], F32, tag="pm")
            for c in range(NC_):
                nc.tensor.matmul(
                    pv[:, 0:BPC],
                    w16[:, c, 128 * k : 128 * (k + 1)],
                    vecT16[:, c, :],
                    start=(c == 0),
                    stop=(c == NC_ - 1),
                )
            nc.vector.tensor_copy(pv_sb[:, k, :], pv[:, 0:BPC])

        # ---------------- main loop ----------------
        for b in [bb for _ in range(passes) for bb in range(BPC)]:
            scores = row_p.tile([1, R], F32, tag="scores")
            nc.vector.memset(scores[:], NEG)
            mask_sb = big.tile([1, R], I32, tag="big")
            nc.sync.dma_start(mask_sb[:], mask_in[b : b + 1, :])

            for rb in range(NBLK):
                r0 = rb * RBLK
                # load 1024 rows of matrix (fp32): [p, t, d]
                m32 = big.tile([128, NT, D], F32, tag="big")
                nc.sync.dma_start(
                    m32[:], mat_in[b, r0 : r0 + RBLK, :].rearrange(
                        "(t p) d -> p t d", p=128))

                # transpose to [d, r] fp16: matT [p, c, r]
                matT = matT_p.tile([128, NC_, RBLK], F16, tag="matT")
                for c in range(NC_):
                    for tg in range(NT // 4):
                        tp = tp_ps.tile([128, 512], F32, tag="tp")
                        for i in range(4):
                            t = tg * 4 + i
                            nc.tensor.transpose(
                                tp[:, 128 * i : 128 * (i + 1)],
                                m32[:, t, 128 * c : 128 * (c + 1)],
                                ident[:],
                            )
                        nc.vector.tensor_copy(
                            matT[:, c, 512 * tg : 512 * (tg + 1)], tp[:]
                        )

                # per e-chunk: proj_m -> tanh -> v-dot
                sch = [sc_ps.tile([1, 512], F32, tag="sc", name=f"sc_{b}_{rb}_{j}")
                       for j in range(2)]
                for k in range(NC_):
                    pm = pm_ps.tile([128, RBLK], F32, tag="pm")
                    for c in range(NC_):
                        for j in range(2):
                            nc.tensor.matmul(
                                pm[:, 512 * j : 512 * (j + 1)],
                                u16[:, c, 128 * k : 128 * (k + 1)],
                                matT[:, c, 512 * j : 512 * (j + 1)],
                                start=(c == 0),
                                stop=(c == NC_ - 1),
                            )
                    inter = inter_p.tile([128, RBLK], F16, tag="inter")
                    nc.scalar.activation(
                        inter[:], pm[:], mybir.ActivationFunctionType.Tanh,
                        bias=pv_sb[:, k, b : b + 1], scale=1.0,
                    )
                    for j in range(2):
                        nc.tensor.matmul(
                            sch[j][:],
                            v16[:, k : k + 1],
                            inter[:, 512 * j : 512 * (j + 1)],
                            start=(k == 0),
                            stop=(k == NC_ - 1),
                        )
                # masked copy into scores row (background is NEG)
                for j in range(2):
                    nc.vector.copy_predicated(
                        scores[:, r0 + 512 * j : r0 + 512 * (j + 1)],
                        mask_sb[:, r0 + 512 * j : r0 + 512 * (j + 1)],
                        sch[j][:],
                    )

            # softmax over the row
            ex = row_p.tile([1, R], F32, tag="ex")
            ssum = consts.tile([1, 1], F32, tag="ssum")
            nc.scalar.activation(
                ex[:], scores[:], mybir.ActivationFunctionType.Exp,
                bias=0.0, scale=1.0, accum_out=ssum[:],
            )
            rec = consts.tile([1, 1], F32, tag="rec")
            nc.vector.reciprocal(rec[:], ssum[:])
            nc.vector.tensor_scalar_mul(ex[:], ex[:], rec[:])
            nc.sync.dma_start(out[b : b + 1, :], ex[:])

    return nc


_NC_CACHE = None


def _get_nc():
    global _NC_CACHE
    if _NC_CACHE is None:
        nc = bass.Bass("TRN2", target_bir_lowering=False, debug=False)
        _emit(nc)
        _legalize_waits(nc)
        _NC_CACHE = nc
    return _NC_CACHE


def kernel(vector, matrix, matrix_mask, w_matrix, u_matrix, v_vector):
    vector = np.ascontiguousarray(np.asarray(vector, dtype=np.float32))
    matrix = np.ascontiguousarray(np.asarray(matrix, dtype=np.float32))
    matrix_mask = np.ascontiguousarray(np.asarray(matrix_mask, dtype=np.int32))
    w_matrix = np.ascontiguousarray(np.asarray(w_matrix, dtype=np.float32))
    u_matrix = np.ascontiguousarray(np.asarray(u_matrix, dtype=np.float32))
    v_vector = np.ascontiguousarray(np.asarray(v_vector, dtype=np.float32))

    nc = _get_nc()
    ident = np.eye(128, dtype=np.float32)
    in_maps = []
    for c in range(NCORES):
        s = slice(c * BPC, (c + 1) * BPC)
        in_maps.append({
            "vec": vector[s],
            "mat": matrix[s],
            "mask": matrix_mask[s],
            "w": w_matrix,
            "u": u_matrix,
            "v": v_vector,
            "ident": ident,
        })
    res = bass_utils.run_bass_kernel_spmd(nc, in_maps, core_ids=list(range(NCORES)))
    return np.concatenate([res.results[c]["out"] for c in range(NCORES)], axis=0)
